# revision 54
# baseline (speedup 1.0000x reference)
"""GroupedQueryAttention Trainium2 kernel.

Sharding: 8 cores = 2 (batch) x 4 (kv-head groups / tensor parallel).
Core c: b = c//4, g = c%4 owns q-heads 4g..4g+3 and kv-head g.
Each core computes a partial o-projection (its 512 rows of Wo); the host
sums the 4 partials per batch (the "all-reduce" of the TP group).

Device kernel per core (all matmuls fp32r, full speed at N>=256):
  1. proj per 512-token chunk: qT/kT/vT = W^T @ x^T in [head_dim, T]
     layout from a host-pretransposed x^T input. RoPE is applied during
     the PSUM->SBUF evacuation using partition-sliced reads (no shift
     DMAs); v is PE-transposed in place back to natural [s, d] layout.
  2. attention interleaved per chunk: S^T[s,t] blocks computed directly
     (lhsT = kT block, rhs = qT chunk) so no P transposes are needed.
     Causal mask added on the diagonal 128-block, exp without max
     subtraction (scores are bounded; exp is safe in fp32).
  3. softmax denominator: per 128-query tile, ones-vector matmuls
     accumulate sum_s P^T[s,t] into a [128,1] PSUM column (ap_size=1,
     nearly free on PE). Reciprocal + DMA round-trip broadcasts 1/den
     to [128, 512], fused into the AV PSUM evacuation multiply.
  4. AV accumulates O^T[d, t-chunk] over s-blocks at N=512.
  5. o-proj: y_partial = O^T^T @ Wo_shard, accumulated over the 4 heads,
     evacuated alternately on DVE/Act, DMA'd straight to DRAM.
"""

import math
import sys

import numpy as np

sys.path.insert(0, "/opt/trn_rl_repo")

import concourse.bass as bass  # noqa: E402
import concourse.tile as tile  # noqa: E402
from concourse import bacc, mybir  # noqa: E402
from concourse.bass_utils import run_bass_kernel_spmd  # noqa: E402

B, T, D = 2, 2048, 2048
NH, NKV, HD = 16, 4, 128
NQ = NH // NKV  # q heads per core
KC = D // 128  # contraction chunks
NT = T // 128  # t tiles
NJ = T // 512  # t chunks
F32 = mybir.dt.float32
F32R = mybir.dt.float32r
X = mybir.AxisListType.X
EXP = mybir.ActivationFunctionType.Exp
COPY = mybir.ActivationFunctionType.Copy
NEGINF = -1.0e30


def _r(ap):
    return ap.bitcast(F32R)


def _body(tc, xt, wq, wk, wv, wo, cost_d, sint_d, maskT_d, identd, swapd_d, y_d):
    nc = tc.nc
    from contextlib import ExitStack

    with ExitStack() as ctx:
        consts = ctx.enter_context(tc.tile_pool(name="consts", bufs=1))
        wpool = ctx.enter_context(tc.tile_pool(name="wpool", bufs=6))
        seq = ctx.enter_context(tc.tile_pool(name="seq", bufs=1))
        blk = ctx.enter_context(tc.tile_pool(name="blk", bufs=17))
        ptp = ctx.enter_context(tc.tile_pool(name="ptp", bufs=5))
        rt = ctx.enter_context(tc.tile_pool(name="rt", bufs=2))
        invp = ctx.enter_context(tc.tile_pool(name="invp", bufs=3))
        dram = ctx.enter_context(tc.tile_pool(name="dram", bufs=2, space="DRAM"))
        ps = ctx.enter_context(tc.tile_pool(name="ps", bufs=3, space="PSUM"))

        # Small consts first (cheap), then weights/x in first-use order so the
        # PE can start as soon as wk/wv/wq0 + the first x tiles land.
        swapid = consts.tile([128, 128], F32R, tag="swapid")
        ident = consts.tile([128, 128], F32R, tag="ident")
        maskT = consts.tile([128, 128], F32, tag="maskT")
        onesr = consts.tile([128, 2], F32, tag="onesr")
        nc.vector.memset(onesr, 1.0)

        # Chunk-0 operands stream in kc-group order (wk/wv pieces + x tiles)
        # so the interleaved k/v/q0 chains start ~3us in and stay fed.
        wkt = wpool.tile([128, 16, 128], F32R, tag="w", name="wkt")
        wvt = wpool.tile([128, 16, 128], F32R, tag="w", name="wvt")
        wqt = [
            wpool.tile([128, 4, 512], F32R, tag="w", name=f"wq{i}") for i in range(4)
        ]
        wkr = wk.rearrange("(c p) m -> p c m", p=128)
        wvr = wv.rearrange("(c p) m -> p c m", p=128)
        xcur = [
            blk.tile([128, 512], F32R, tag="blk", name=f"xt0_{kc}")
            for kc in range(KC)
        ]
        cost = consts.tile([128, T], F32, tag="cost")
        sint = consts.tile([128, T], F32, tag="sint")
        for g in range(4):
            nc.sync.dma_start(wkt[:, 4 * g : 4 * g + 4, :], wkr[:, 4 * g : 4 * g + 4, :])
            if g == 0:
                nc.sync.dma_start(xcur[0], xt[0:128, 0:512])
            nc.sync.dma_start(wvt[:, 4 * g : 4 * g + 4, :], wvr[:, 4 * g : 4 * g + 4, :])
            # wq arrives as per-kc sub-tiles interleaved with the x stream so
            # the skewed q chains never wait on a whole-tile wq load
            wqr = wq[512 * g : 512 * (g + 1), :].rearrange("(c p) m -> p c m", p=128)
            for i, kc in enumerate(range(4 * g, 4 * g + 4)):
                if not (g == 0 and kc == 0):
                    nc.sync.dma_start(xcur[kc], xt[128 * kc : 128 * (kc + 1), 0:512])
                nc.sync.dma_start(wqt[g][:, i : i + 1, :], wqr[:, i : i + 1, :])
        nc.sync.dma_start(swapid, swapd_d)
        nc.sync.dma_start(ident, identd)
        nc.sync.dma_start(maskT, maskT_d)
        nc.sync.dma_start(cost[:, 0:512], cost_d[:, 0:512])
        nc.sync.dma_start(sint[:, 0:512], sint_d[:, 0:512])

        qT = [seq.tile([128, T], F32R, tag=f"qT{h}", name=f"qT{h}") for h in range(NQ)]
        OT = [seq.tile([128, T], F32R, tag=f"ot{h}", name=f"ot{h}") for h in range(NQ)]
        kT = seq.tile([128, T], F32R, tag="kT", name="kT")
        vnat = seq.tile([128, T], F32R, tag="vnat", name="vnat")
        wot = []

        def wslc(m, kc):
            if m == "k":
                return wkt[:, kc, :]
            if m == "v":
                return wvt[:, kc, :]
            h = int(m[1])
            return wqt[kc // 4][:, kc % 4, 128 * h : 128 * (h + 1)]

        # ---------- emission units (proj chains, evacs, o-proj tiles) ------
        def chain_part(j, m, ref, xts, lo, hi):
            def emit():
                if lo == 0:
                    ref["pm"] = ps.tile(
                        [128, 512], F32, tag="pm", bufs=2, name=f"pm{j}_{m}"
                    )
                for kc in range(lo, hi):
                    nc.tensor.matmul(
                        ref["pm"],
                        _r(wslc(m, kc)),
                        _r(xts[kc]),
                        start=(kc == 0),
                        stop=(kc == KC - 1),
                    )
            return emit

        def rope_evac(j, m, ref, act=False):
            def emit():
                ch = slice(512 * j, 512 * (j + 1))
                pm = ref["pm"]
                tgt = kT if m == "k" else qT[int(m[1])]
                if act:
                    nc.scalar.activation(tgt[:, ch], pm, COPY)
                else:
                    nc.vector.tensor_copy(tgt[:, ch], pm)
                rot = ps.tile([128, 512], F32, tag="pm", bufs=2, name=f"rot{j}_{m}")
                nc.tensor.matmul(rot, _r(swapid), _r(tgt[:, ch]))
                nc.gpsimd.tensor_mul(tgt[:, ch], tgt[:, ch], cost[:, ch])
                tmp = rt.tile([128, 512], F32, tag="yst", bufs=5, name=f"rt{j}_{m}")
                nc.vector.tensor_mul(tmp, rot, sint[:, ch])
                nc.vector.tensor_add(tgt[:, ch], tgt[:, ch], tmp)
            return emit

        def v_evac(j, ref):
            def emit():
                vtmp = blk.tile([128, 512], F32R, tag="blk", name=f"vtmp{j}")
                nc.scalar.activation(vtmp, ref["pm"], COPY)
                ref["vtmp"] = vtmp
            return emit

        def v_transpose(j, ref):
            def emit():
                ch = slice(512 * j, 512 * (j + 1))
                pmv, vtmp = ref["pm"], ref["vtmp"]
                for c in range(4):
                    nc.tensor.matmul(
                        _r(pmv[:, 128 * c : 128 * (c + 1)]),
                        _r(vtmp[:, 128 * c : 128 * (c + 1)]),
                        _r(ident),
                        is_transpose=True,
                        start=(c == 0),
                        stop=(c == 3),
                    )
                nc.vector.tensor_copy(vnat[:, ch], pmv)
            return emit

        def wo_load(hh):
            def emit():
                w = wpool.tile([128, T], F32R, tag="w", name=f"wo{hh}")
                nc.sync.dma_start(w, wo[128 * hh : 128 * (hh + 1), :])
                wot.append(w)
            return emit

        def oproj_tile(it, nch, inj=False):
            def emit():
                yp = ps.tile([128, 512], F32, tag="pm", bufs=2, name=f"yp{it}_{nch}")
                for hh in range(4):
                    nc.tensor.matmul(
                        yp,
                        _r(OT[hh][:, 128 * it : 128 * (it + 1)]),
                        _r(wot[hh][:, 512 * nch : 512 * (nch + 1)]),
                        start=(hh == 0),
                        stop=(hh == 3),
                    )
                yst = rt.tile([128, 512], F32, tag="yst", bufs=5, name=f"yst{it}_{nch}")
                if nch % 2 == 0 and not inj:
                    nc.scalar.activation(yst, yp, COPY)
                else:
                    nc.vector.tensor_copy(yst, yp)
                nc.sync.dma_start(
                    y_d[128 * it : 128 * (it + 1), 512 * nch : 512 * (nch + 1)], yst
                )
            return emit

        def proj_units(j, xts):
            units = []
            for m in ["k", "v", "q0", "q1", "q2", "q3"]:
                ref = {}
                for lo in range(0, KC, 4):
                    units.append(chain_part(j, m, ref, xts, lo, lo + 4))
                if m == "v":
                    units.append(v_evac(j, ref))
                    units.append(v_transpose(j, ref))
                else:
                    units.append(rope_evac(j, m, ref))
                # during attention(2) we emit proj(3); hook the wo loads in
                # as their wpool ring slots free up (wo0<-wkt, wo1<-wvt,
                # wo2<-wq0, wo3<-wq1)
                if j == 3:
                    if m == "k":
                        units.append(wo_load(0))
                    elif m == "v":
                        units.append(wo_load(1))
                    elif m == "q0":
                        units.append(wo_load(2))
                    elif m == "q1":
                        units.append(wo_load(3))
            return units

        # ---------- prologue: proj(0) with all six chains interleaved so the
        # PE tracks the x/w stream-in; q1/q2/q3 borrow the idle av/den PSUM
        # tags (attention hasn't started yet; ring WARs resolve cleanly)
        MS = ["k", "v", "q0", "q1", "q2", "q3"]
        QS = ["q0", "q1", "q2", "q3"]
        ptag = {"k": ("pm", 2), "v": ("pm", 2), "q0": ("ps", 3),
                "q1": ("av", 2), "q2": ("av", 2), "q3": ("den", 1)}
        refs = {m: {} for m in MS}
        for m in MS:
            tg, nb = ptag[m]
            refs[m]["pm"] = ps.tile(
                [128, 512], F32, tag=tg, bufs=nb, name=f"pm0_{m}"
            )

        def mm0(m, kc):
            nc.tensor.matmul(
                refs[m]["pm"],
                _r(wslc(m, kc)),
                _r(xcur[kc]),
                start=(kc == 0),
                stop=(kc == KC - 1),
            )

        # q chains lag k/v by 4 kc rounds so the PE can start on k/v while
        # wq0 is still in flight
        for kc in range(KC):
            mm0("k", kc)
            mm0("v", kc)
            if kc >= 4:
                for m in QS:
                    mm0(m, kc - 4)
        for kc in range(KC - 4, KC):
            for m in QS:
                mm0(m, kc)
        rope_evac(0, "k", refs["k"], act=True)()
        rope_evac(0, "q0", refs["q0"])()
        v_evac(0, refs["v"])()
        v_transpose(0, refs["v"])()
        pro_evacs = [
            rope_evac(0, "q1", refs["q1"], act=True),
            rope_evac(0, "q2", refs["q2"]),
            rope_evac(0, "q3", refs["q3"], act=True),
        ]

        # ---------- attention per chunk, feeder interleaves next-chunk work
        for j in range(NJ):
            ch = slice(512 * j, 512 * (j + 1))
            nst = 4 * j + 4

            # stage next chunk's x tiles + build the feeder
            if j < 3:
                sl = slice(512 * (j + 1), 512 * (j + 2))
                nc.sync.dma_start(cost[:, sl], cost_d[:, sl])
                nc.sync.dma_start(sint[:, sl], sint_d[:, sl])
                xnext = []
                for kc in range(KC):
                    xtile = blk.tile(
                        [128, 512], F32R, tag="blk", name=f"xt{j + 1}_{kc}"
                    )
                    nc.sync.dma_start(
                        xtile, xt[128 * kc : 128 * (kc + 1), 512 * (j + 1) : 512 * (j + 2)]
                    )
                    xnext.append(xtile)
                feed = proj_units(j + 1, xnext)
                if j == 0:
                    feed = pro_evacs + feed
            else:
                feed = [
                    oproj_tile(it, nch, inj=True)
                    for it in range(12)
                    for nch in range(4)
                ]

            def inject(n):
                for _ in range(n):
                    if feed:
                        feed.pop(0)()

            for h in range(NQ):
                den8 = ps.tile(
                    [128, 8],
                    F32,
                    tag="den",
                    bufs=1,
                    padded_shape=[128, 512],
                    name=f"den{h}_{j}",
                )
                av = ps.tile([128, 512], F32, tag="av", bufs=2, name=f"av{h}_{j}")
                pts = [None] * nst

                def s_block(st):
                    off = 128 * (st - 4 * j)
                    # last diagonal block: compute 256 wide (fp32r needs >=256
                    # for 1 cyc/row); extended mask zeroes the extra columns
                    lo = 256 if off == 384 else max(0, off)
                    sps = ps.tile([128, 512], F32, tag="ps", name=f"s{h}_{j}_{st}")
                    nc.tensor.matmul(
                        sps[:, lo:512],
                        _r(kT[:, 128 * st : 128 * (st + 1)]),
                        _r(qT[h][:, 512 * j + lo : 512 * (j + 1)]),
                    )
                    if off >= 0:
                        nc.vector.tensor_add(
                            sps[:, off : off + 128], sps[:, off : off + 128], maskT
                        )
                    pt = ptp.tile([128, 512], F32R, tag="pt", name=f"pt{h}_{j}_{st}")
                    if off == 384:
                        # dead cols of the widened last block: zeroed early on
                        # the idle gpsimd so AV can run 256 wide gated only on
                        # the (narrowed) exp
                        nc.gpsimd.memset(pt[:, 256:384].bitcast(F32), 0.0)
                        nc.scalar.activation(pt[:, 384:512], sps[:, 384:512], EXP)
                    else:
                        nc.scalar.activation(pt[:, lo:512], sps[:, lo:512], EXP)
                    pts[st] = pt

                def den_av(st):
                    # denominator: four column chains form ONE psum group
                    for c in range(max(0, st - 4 * j), 4):
                        nc.tensor.matmul(
                            den8[:, 2 * c : 2 * c + 2],
                            _r(pts[st][:, 128 * c : 128 * (c + 1)]),
                            _r(onesr),
                            start=(st == 0 and c == 0),
                            stop=(st == nst - 1 and c == 3),
                        )
                    c0 = 256 if st - 4 * j == 3 else max(0, 128 * (st - 4 * j))
                    nc.tensor.matmul(
                        av[:, c0:512],
                        _r(vnat[:, 128 * st : 128 * (st + 1)]),
                        _r(pts[st][:, c0:512]),
                        start=(st == 0),
                        stop=(st == nst - 1),
                    )

                # den/av lag the S^T/exp stream by one block so the PE never
                # waits on the Activation engine's exp
                s_block(0)
                if nst > 1:
                    s_block(1)
                for st in range(nst):
                    if st + 2 < nst:
                        s_block(st + 2)
                    if (st % 2 == 1 or st >= nst - 4) and not (j == 3 and h == 0):
                        inject(1)
                    if st >= 1:
                        den_av(st - 1)
                den_av(nst - 1)

                # 1/den broadcast along partitions via DRAM round trip
                den4sb = rt.tile([128, 4], F32, tag="d4", name=f"d4_{h}_{j}")
                nc.vector.reciprocal(den4sb, den8[:, 0:8:2])
                dfd = dram.tile([1, 512], F32, tag="dfd", name=f"dfd{h}_{j}")
                nc.sync.dma_start(dfd.rearrange("a (c p) -> p a c", p=128), den4sb)
                inv_b = invp.tile([128, 512], F32, tag="inv", name=f"inv{h}_{j}")
                nc.gpsimd.dma_start(inv_b, dfd[0:1, :].to_broadcast([128, 512]))
                nc.vector.tensor_mul(OT[h][:, ch], av, inv_b)

                if not (j == 3 and h == 0):
                    inject(4)

            # drain this chunk's feeder before the next chunk needs it
            inject(len(feed))

        # ---------- o-proj tail (t-tiles 12..15) ----------
        for it in range(12, NT):
            for nch in range(4):
                oproj_tile(it, nch)()


def build_nc():
    nc = bacc.Bacc("TRN2", target_bir_lowering=False, debug=False, num_devices=8)
    xt = nc.dram_tensor("xt", [D, T], F32R, kind="ExternalInput").ap()
    wq = nc.dram_tensor("wq", [D, NQ * HD], F32R, kind="ExternalInput").ap()
    wk = nc.dram_tensor("wk", [D, HD], F32R, kind="ExternalInput").ap()
    wv = nc.dram_tensor("wv", [D, HD], F32R, kind="ExternalInput").ap()
    wo = nc.dram_tensor("wo", [NQ * HD, D], F32R, kind="ExternalInput").ap()
    identd = nc.dram_tensor("identd", [128, 128], F32R, kind="ExternalInput").ap()
    swapd = nc.dram_tensor("swapd", [128, 128], F32R, kind="ExternalInput").ap()
    cost = nc.dram_tensor("cost", [HD, T], F32, kind="ExternalInput").ap()
    sint = nc.dram_tensor("sint", [HD, T], F32, kind="ExternalInput").ap()
    maskT = nc.dram_tensor("maskT", [128, 128], F32, kind="ExternalInput").ap()
    y = nc.dram_tensor("y", [T, D], F32, kind="ExternalOutput").ap()
    with tile.TileContext(nc) as tc:
        _body(tc, xt, wq, wk, wv, wo, cost, sint, maskT, identd, swapd, y)
    nc.compile()
    return nc


def rope_tables():
    inv_freq = 1.0 / (10000.0 ** (np.arange(0, HD, 2, dtype=np.float32) / HD))
    t = np.arange(T, dtype=np.float32)
    freqs = t[:, None] * inv_freq[None, :]
    emb = np.concatenate([freqs, freqs], axis=1)  # [T, 128]
    cos = np.ascontiguousarray(np.cos(emb).T).astype(np.float32)
    sin = np.ascontiguousarray(np.sin(emb).T).astype(np.float32)
    sins = sin.copy()
    sins[0:64] = -sins[0:64]
    return cos, sins


def causal_mask_tile():
    # S^T layout: rows = s, cols = t; valid (0.0) where s <= t.
    tt = np.arange(128)
    return np.where(tt[:, None] <= tt[None, :], 0.0, NEGINF).astype(np.float32)


def half_swap_tile():
    # lhsT for rotate_half: out[m] = in[(m + 64) % 128] (sign folded in sint)
    sw = np.zeros((128, 128), dtype=np.float32)
    sw[(np.arange(128) + 64) % 128, np.arange(128)] = 1.0
    return sw


def make_in_maps(x, Wq, Wk, Wv, Wo):
    scale = np.float32(1.0 / math.sqrt(HD))
    cos, sins = rope_tables()
    mask = causal_mask_tile()
    in_maps = []
    for c in range(8):
        b, g = c // 4, c % 4
        in_maps.append(
            {
                "xt": np.ascontiguousarray(x[b].T),
                "wq": np.ascontiguousarray(Wq[:, 512 * g : 512 * (g + 1)]) * scale,
                "wk": np.ascontiguousarray(Wk[:, 128 * g : 128 * (g + 1)]),
                "wv": np.ascontiguousarray(Wv[:, 128 * g : 128 * (g + 1)]),
                "wo": np.ascontiguousarray(Wo[512 * g : 512 * (g + 1), :]),
                "cost": cos,
                "sint": sins,
                "maskT": mask,
                "identd": np.eye(128, dtype=np.float32),
                "swapd": half_swap_tile(),
            }
        )
    return in_maps


_CACHE = {}


def _get_nc():
    if "nc" not in _CACHE:
        _CACHE["nc"] = build_nc()
    return _CACHE["nc"]


def kernel(**inputs):
    x = np.asarray(inputs["x"], np.float32)
    Wq = np.asarray(inputs["Wq"], np.float32)
    Wk = np.asarray(inputs["Wk"], np.float32)
    Wv = np.asarray(inputs["Wv"], np.float32)
    Wo = np.asarray(inputs["Wo"], np.float32)
    in_maps = make_in_maps(x, Wq, Wk, Wv, Wo)
    nc = _get_nc()
    res = run_bass_kernel_spmd(nc, in_maps, core_ids=list(range(8)))
    outs = [r["y"] for r in res.results]
    y = np.stack(
        [
            outs[0] + outs[1] + outs[2] + outs[3],
            outs[4] + outs[5] + outs[6] + outs[7],
        ]
    )
    return y.astype(np.float32)


# revision 55
# speedup vs baseline: 1.0037x; 1.0037x over previous
"""GroupedQueryAttention Trainium2 kernel.

Sharding: 8 cores = 2 (batch) x 4 (kv-head groups / tensor parallel).
Core c: b = c//4, g = c%4 owns q-heads 4g..4g+3 and kv-head g.
Each core computes a partial o-projection (its 512 rows of Wo); the host
sums the 4 partials per batch (the "all-reduce" of the TP group).

Device kernel per core (all matmuls fp32r, full speed at N>=256):
  1. proj per 512-token chunk: qT/kT/vT = W^T @ x^T in [head_dim, T]
     layout from a host-pretransposed x^T input. RoPE is applied during
     the PSUM->SBUF evacuation using partition-sliced reads (no shift
     DMAs); v is PE-transposed in place back to natural [s, d] layout.
  2. attention interleaved per chunk: S^T[s,t] blocks computed directly
     (lhsT = kT block, rhs = qT chunk) so no P transposes are needed.
     Causal mask added on the diagonal 128-block, exp without max
     subtraction (scores are bounded; exp is safe in fp32).
  3. softmax denominator: per 128-query tile, ones-vector matmuls
     accumulate sum_s P^T[s,t] into a [128,1] PSUM column (ap_size=1,
     nearly free on PE). Reciprocal + DMA round-trip broadcasts 1/den
     to [128, 512], fused into the AV PSUM evacuation multiply.
  4. AV accumulates O^T[d, t-chunk] over s-blocks at N=512.
  5. o-proj: y_partial = O^T^T @ Wo_shard, accumulated over the 4 heads,
     evacuated alternately on DVE/Act, DMA'd straight to DRAM.
"""

import math
import sys

import numpy as np

sys.path.insert(0, "/opt/trn_rl_repo")

import concourse.bass as bass  # noqa: E402
import concourse.tile as tile  # noqa: E402
from concourse import bacc, mybir  # noqa: E402
from concourse.bass_utils import run_bass_kernel_spmd  # noqa: E402

B, T, D = 2, 2048, 2048
NH, NKV, HD = 16, 4, 128
NQ = NH // NKV  # q heads per core
KC = D // 128  # contraction chunks
NT = T // 128  # t tiles
NJ = T // 512  # t chunks
F32 = mybir.dt.float32
F32R = mybir.dt.float32r
X = mybir.AxisListType.X
EXP = mybir.ActivationFunctionType.Exp
COPY = mybir.ActivationFunctionType.Copy
NEGINF = -1.0e30


def _r(ap):
    return ap.bitcast(F32R)


def _body(tc, xt, wq, wk, wv, wo, cost_d, sint_d, maskT_d, identd, swapd_d, y_d):
    nc = tc.nc
    from contextlib import ExitStack

    with ExitStack() as ctx:
        consts = ctx.enter_context(tc.tile_pool(name="consts", bufs=1))
        wpool = ctx.enter_context(tc.tile_pool(name="wpool", bufs=6))
        seq = ctx.enter_context(tc.tile_pool(name="seq", bufs=1))
        blk = ctx.enter_context(tc.tile_pool(name="blk", bufs=17))
        ptp = ctx.enter_context(tc.tile_pool(name="ptp", bufs=5))
        rt = ctx.enter_context(tc.tile_pool(name="rt", bufs=2))
        invp = ctx.enter_context(tc.tile_pool(name="invp", bufs=3))
        dram = ctx.enter_context(tc.tile_pool(name="dram", bufs=2, space="DRAM"))
        ps = ctx.enter_context(tc.tile_pool(name="ps", bufs=3, space="PSUM"))

        # Small consts first (cheap), then weights/x in first-use order so the
        # PE can start as soon as wk/wv/wq0 + the first x tiles land.
        swapid = consts.tile([128, 128], F32R, tag="swapid")
        ident = consts.tile([128, 128], F32R, tag="ident")
        maskT = consts.tile([128, 128], F32, tag="maskT")
        onesr = consts.tile([128, 2], F32, tag="onesr")
        nc.vector.memset(onesr, 1.0)

        # Chunk-0 operands stream in kc-group order (wk/wv pieces + x tiles)
        # so the interleaved k/v/q0 chains start ~3us in and stay fed.
        wkt = wpool.tile([128, 16, 128], F32R, tag="w", name="wkt")
        wvt = wpool.tile([128, 16, 128], F32R, tag="w", name="wvt")
        wqt = [
            wpool.tile([128, 4, 512], F32R, tag="w", name=f"wq{i}") for i in range(4)
        ]
        wkr = wk.rearrange("(c p) m -> p c m", p=128)
        wvr = wv.rearrange("(c p) m -> p c m", p=128)
        xcur = [
            blk.tile([128, 512], F32R, tag="blk", name=f"xt0_{kc}")
            for kc in range(KC)
        ]
        cost = consts.tile([128, T], F32, tag="cost")
        sint = consts.tile([128, T], F32, tag="sint")
        for g in range(4):
            nc.sync.dma_start(wkt[:, 4 * g : 4 * g + 4, :], wkr[:, 4 * g : 4 * g + 4, :])
            if g == 0:
                nc.sync.dma_start(xcur[0], xt[0:128, 0:512])
            nc.sync.dma_start(wvt[:, 4 * g : 4 * g + 4, :], wvr[:, 4 * g : 4 * g + 4, :])
            # wq arrives as per-kc sub-tiles interleaved with the x stream so
            # the skewed q chains never wait on a whole-tile wq load
            wqr = wq[512 * g : 512 * (g + 1), :].rearrange("(c p) m -> p c m", p=128)
            for i, kc in enumerate(range(4 * g, 4 * g + 4)):
                if not (g == 0 and kc == 0):
                    nc.sync.dma_start(xcur[kc], xt[128 * kc : 128 * (kc + 1), 0:512])
                nc.sync.dma_start(wqt[g][:, i : i + 1, :], wqr[:, i : i + 1, :])
        nc.sync.dma_start(swapid, swapd_d)
        nc.sync.dma_start(ident, identd)
        nc.sync.dma_start(maskT, maskT_d)
        nc.sync.dma_start(cost[:, 0:512], cost_d[:, 0:512])
        nc.sync.dma_start(sint[:, 0:512], sint_d[:, 0:512])

        qT = [seq.tile([128, T], F32R, tag=f"qT{h}", name=f"qT{h}") for h in range(NQ)]
        OT = [seq.tile([128, T], F32R, tag=f"ot{h}", name=f"ot{h}") for h in range(NQ)]
        kT = seq.tile([128, T], F32R, tag="kT", name="kT")
        vnat = seq.tile([128, T], F32R, tag="vnat", name="vnat")
        wot = []

        def wslc(m, kc):
            if m == "k":
                return wkt[:, kc, :]
            if m == "v":
                return wvt[:, kc, :]
            h = int(m[1])
            return wqt[kc // 4][:, kc % 4, 128 * h : 128 * (h + 1)]

        # ---------- emission units (proj chains, evacs, o-proj tiles) ------
        def chain_part(j, m, ref, xts, lo, hi):
            def emit():
                if lo == 0:
                    ref["pm"] = ps.tile(
                        [128, 512], F32, tag="pm", bufs=2, name=f"pm{j}_{m}"
                    )
                for kc in range(lo, hi):
                    nc.tensor.matmul(
                        ref["pm"],
                        _r(wslc(m, kc)),
                        _r(xts[kc]),
                        start=(kc == 0),
                        stop=(kc == KC - 1),
                    )
            return emit

        def rope_evac(j, m, ref, act=False):
            def emit():
                ch = slice(512 * j, 512 * (j + 1))
                pm = ref["pm"]
                tgt = kT if m == "k" else qT[int(m[1])]
                if act:
                    nc.scalar.activation(tgt[:, ch], pm, COPY)
                else:
                    nc.vector.tensor_copy(tgt[:, ch], pm)
                rot = ps.tile([128, 512], F32, tag="pm", bufs=2, name=f"rot{j}_{m}")
                nc.tensor.matmul(rot, _r(swapid), _r(tgt[:, ch]))
                nc.gpsimd.tensor_mul(tgt[:, ch], tgt[:, ch], cost[:, ch])
                tmp = rt.tile([128, 512], F32, tag="yst", bufs=5, name=f"rt{j}_{m}")
                nc.vector.tensor_mul(tmp, rot, sint[:, ch])
                nc.vector.tensor_add(tgt[:, ch], tgt[:, ch], tmp)
            return emit

        def v_evac(j, ref):
            def emit():
                vtmp = blk.tile([128, 512], F32R, tag="blk", name=f"vtmp{j}")
                nc.scalar.activation(vtmp, ref["pm"], COPY)
                ref["vtmp"] = vtmp
            return emit

        def v_transpose(j, ref):
            def emit():
                ch = slice(512 * j, 512 * (j + 1))
                pmv, vtmp = ref["pm"], ref["vtmp"]
                for c in range(4):
                    nc.tensor.matmul(
                        _r(pmv[:, 128 * c : 128 * (c + 1)]),
                        _r(vtmp[:, 128 * c : 128 * (c + 1)]),
                        _r(ident),
                        is_transpose=True,
                        start=(c == 0),
                        stop=(c == 3),
                    )
                nc.vector.tensor_copy(vnat[:, ch], pmv)
            return emit

        def wo_load(hh):
            def emit():
                w = wpool.tile([128, T], F32R, tag="w", name=f"wo{hh}")
                nc.sync.dma_start(w, wo[128 * hh : 128 * (hh + 1), :])
                wot.append(w)
            return emit

        def oproj_tile(it, nch, inj=False):
            def emit():
                yp = ps.tile([128, 512], F32, tag="pm", bufs=2, name=f"yp{it}_{nch}")
                for hh in range(4):
                    nc.tensor.matmul(
                        yp,
                        _r(OT[hh][:, 128 * it : 128 * (it + 1)]),
                        _r(wot[hh][:, 512 * nch : 512 * (nch + 1)]),
                        start=(hh == 0),
                        stop=(hh == 3),
                    )
                yst = rt.tile([128, 512], F32, tag="yst", bufs=5, name=f"yst{it}_{nch}")
                if nch % 2 == 0 and not inj:
                    nc.scalar.activation(yst, yp, COPY)
                else:
                    nc.vector.tensor_copy(yst, yp)
                nc.sync.dma_start(
                    y_d[128 * it : 128 * (it + 1), 512 * nch : 512 * (nch + 1)], yst
                )
            return emit

        def proj_units(j, xts):
            units = []
            for m in ["k", "v", "q0", "q1", "q2", "q3"]:
                ref = {}
                for lo in range(0, KC, 4):
                    units.append(chain_part(j, m, ref, xts, lo, lo + 4))
                if m == "v":
                    units.append(v_evac(j, ref))
                    units.append(v_transpose(j, ref))
                else:
                    units.append(rope_evac(j, m, ref))
                # during attention(2) we emit proj(3); hook the wo loads in
                # as their wpool ring slots free up (wo0<-wkt, wo1<-wvt,
                # wo2<-wq0, wo3<-wq1)
                if j == 3:
                    if m == "k":
                        units.append(wo_load(0))
                    elif m == "v":
                        units.append(wo_load(1))
                    elif m == "q0":
                        units.append(wo_load(2))
                    elif m == "q1":
                        units.append(wo_load(3))
            return units

        # ---------- prologue: proj(0) with all six chains interleaved so the
        # PE tracks the x/w stream-in; q1/q2/q3 borrow the idle av/den PSUM
        # tags (attention hasn't started yet; ring WARs resolve cleanly)
        MS = ["k", "v", "q0", "q1", "q2", "q3"]
        QS = ["q0", "q1", "q2", "q3"]
        ptag = {"k": ("pm", 2), "v": ("pm", 2), "q0": ("ps", 3),
                "q1": ("av", 2), "q2": ("av", 2), "q3": ("den", 1)}
        refs = {m: {} for m in MS}
        for m in MS:
            tg, nb = ptag[m]
            refs[m]["pm"] = ps.tile(
                [128, 512], F32, tag=tg, bufs=nb, name=f"pm0_{m}"
            )

        def mm0(m, kc):
            nc.tensor.matmul(
                refs[m]["pm"],
                _r(wslc(m, kc)),
                _r(xcur[kc]),
                start=(kc == 0),
                stop=(kc == KC - 1),
            )

        # q chains lag k/v by 4 kc rounds so the PE can start on k/v while
        # wq0 is still in flight
        for kc in range(KC):
            mm0("k", kc)
            mm0("v", kc)
            if kc >= 4:
                for m in QS:
                    mm0(m, kc - 4)
        for kc in range(KC - 4, KC):
            for m in QS:
                mm0(m, kc)
        rope_evac(0, "k", refs["k"], act=True)()
        rope_evac(0, "q0", refs["q0"])()
        v_evac(0, refs["v"])()
        v_transpose(0, refs["v"])()
        pro_evacs = [
            rope_evac(0, "q1", refs["q1"], act=True),
            rope_evac(0, "q2", refs["q2"]),
            rope_evac(0, "q3", refs["q3"], act=True),
        ]

        # ---------- attention per chunk, feeder interleaves next-chunk work
        for j in range(NJ):
            ch = slice(512 * j, 512 * (j + 1))
            nst = 4 * j + 4

            # stage next chunk's x tiles + build the feeder
            if j < 3:
                sl = slice(512 * (j + 1), 512 * (j + 2))
                nc.sync.dma_start(cost[:, sl], cost_d[:, sl])
                nc.sync.dma_start(sint[:, sl], sint_d[:, sl])
                xnext = []
                for kc in range(KC):
                    xtile = blk.tile(
                        [128, 512], F32R, tag="blk", name=f"xt{j + 1}_{kc}"
                    )
                    nc.sync.dma_start(
                        xtile, xt[128 * kc : 128 * (kc + 1), 512 * (j + 1) : 512 * (j + 2)]
                    )
                    xnext.append(xtile)
                feed = proj_units(j + 1, xnext)
                if j == 0:
                    feed = pro_evacs + feed
            else:
                feed = [
                    oproj_tile(it, nch, inj=True)
                    for it in range(12)
                    for nch in range(4)
                ]

            def inject(n):
                for _ in range(n):
                    if feed:
                        feed.pop(0)()

            for h in range(NQ):
                den8 = ps.tile(
                    [128, 8],
                    F32,
                    tag="den",
                    bufs=1,
                    padded_shape=[128, 512],
                    name=f"den{h}_{j}",
                )
                av = ps.tile([128, 512], F32, tag="av", bufs=2, name=f"av{h}_{j}")
                pts = [None] * nst

                def s_block(st):
                    off = 128 * (st - 4 * j)
                    # last diagonal block: compute 256 wide (fp32r needs >=256
                    # for 1 cyc/row); extended mask zeroes the extra columns
                    lo = 256 if off == 384 else max(0, off)
                    sps = ps.tile([128, 512], F32, tag="ps", name=f"s{h}_{j}_{st}")
                    nc.tensor.matmul(
                        sps[:, lo:512],
                        _r(kT[:, 128 * st : 128 * (st + 1)]),
                        _r(qT[h][:, 512 * j + lo : 512 * (j + 1)]),
                    )
                    if off >= 0:
                        nc.vector.tensor_add(
                            sps[:, off : off + 128], sps[:, off : off + 128], maskT
                        )
                    pt = ptp.tile([128, 512], F32R, tag="pt", name=f"pt{h}_{j}_{st}")
                    nc.scalar.activation(pt[:, lo:512], sps[:, lo:512], EXP)
                    if off == 384:
                        # dead cols of the widened block: zero so AV can run
                        # 256 wide (1 cyc/row instead of 128 wide at 4)
                        nc.vector.memset(pt[:, 256:384].bitcast(F32), 0.0)
                    pts[st] = pt

                def den_av(st):
                    # denominator: four column chains form ONE psum group
                    for c in range(max(0, st - 4 * j), 4):
                        nc.tensor.matmul(
                            den8[:, 2 * c : 2 * c + 2],
                            _r(pts[st][:, 128 * c : 128 * (c + 1)]),
                            _r(onesr),
                            start=(st == 0 and c == 0),
                            stop=(st == nst - 1 and c == 3),
                        )
                    c0 = 256 if st - 4 * j == 3 else max(0, 128 * (st - 4 * j))
                    nc.tensor.matmul(
                        av[:, c0:512],
                        _r(vnat[:, 128 * st : 128 * (st + 1)]),
                        _r(pts[st][:, c0:512]),
                        start=(st == 0),
                        stop=(st == nst - 1),
                    )

                # den/av lag the S^T/exp stream by one block so the PE never
                # waits on the Activation engine's exp
                s_block(0)
                if nst > 1:
                    s_block(1)
                for st in range(nst):
                    if st + 2 < nst:
                        s_block(st + 2)
                    if (st % 2 == 1 or st >= nst - 4) and not (j == 3 and h == 0):
                        inject(1)
                    if st >= 1:
                        den_av(st - 1)
                den_av(nst - 1)

                # 1/den broadcast along partitions via DRAM round trip
                den4sb = rt.tile([128, 4], F32, tag="d4", name=f"d4_{h}_{j}")
                nc.vector.reciprocal(den4sb, den8[:, 0:8:2])
                dfd = dram.tile([1, 512], F32, tag="dfd", name=f"dfd{h}_{j}")
                nc.sync.dma_start(dfd.rearrange("a (c p) -> p a c", p=128), den4sb)
                inv_b = invp.tile([128, 512], F32, tag="inv", name=f"inv{h}_{j}")
                nc.gpsimd.dma_start(inv_b, dfd[0:1, :].to_broadcast([128, 512]))
                nc.vector.tensor_mul(OT[h][:, ch], av, inv_b)

                if not (j == 3 and h == 0):
                    inject(4)

            # drain this chunk's feeder before the next chunk needs it
            inject(len(feed))

        # ---------- o-proj tail (t-tiles 12..15) ----------
        for it in range(12, NT):
            for nch in range(4):
                oproj_tile(it, nch)()


def build_nc():
    nc = bacc.Bacc("TRN2", target_bir_lowering=False, debug=False, num_devices=8)
    xt = nc.dram_tensor("xt", [D, T], F32R, kind="ExternalInput").ap()
    wq = nc.dram_tensor("wq", [D, NQ * HD], F32R, kind="ExternalInput").ap()
    wk = nc.dram_tensor("wk", [D, HD], F32R, kind="ExternalInput").ap()
    wv = nc.dram_tensor("wv", [D, HD], F32R, kind="ExternalInput").ap()
    wo = nc.dram_tensor("wo", [NQ * HD, D], F32R, kind="ExternalInput").ap()
    identd = nc.dram_tensor("identd", [128, 128], F32R, kind="ExternalInput").ap()
    swapd = nc.dram_tensor("swapd", [128, 128], F32R, kind="ExternalInput").ap()
    cost = nc.dram_tensor("cost", [HD, T], F32, kind="ExternalInput").ap()
    sint = nc.dram_tensor("sint", [HD, T], F32, kind="ExternalInput").ap()
    maskT = nc.dram_tensor("maskT", [128, 128], F32, kind="ExternalInput").ap()
    y = nc.dram_tensor("y", [T, D], F32, kind="ExternalOutput").ap()
    with tile.TileContext(nc) as tc:
        _body(tc, xt, wq, wk, wv, wo, cost, sint, maskT, identd, swapd, y)
    nc.compile()
    return nc


def rope_tables():
    inv_freq = 1.0 / (10000.0 ** (np.arange(0, HD, 2, dtype=np.float32) / HD))
    t = np.arange(T, dtype=np.float32)
    freqs = t[:, None] * inv_freq[None, :]
    emb = np.concatenate([freqs, freqs], axis=1)  # [T, 128]
    cos = np.ascontiguousarray(np.cos(emb).T).astype(np.float32)
    sin = np.ascontiguousarray(np.sin(emb).T).astype(np.float32)
    sins = sin.copy()
    sins[0:64] = -sins[0:64]
    return cos, sins


def causal_mask_tile():
    # S^T layout: rows = s, cols = t; valid (0.0) where s <= t.
    tt = np.arange(128)
    return np.where(tt[:, None] <= tt[None, :], 0.0, NEGINF).astype(np.float32)


def half_swap_tile():
    # lhsT for rotate_half: out[m] = in[(m + 64) % 128] (sign folded in sint)
    sw = np.zeros((128, 128), dtype=np.float32)
    sw[(np.arange(128) + 64) % 128, np.arange(128)] = 1.0
    return sw


def make_in_maps(x, Wq, Wk, Wv, Wo):
    scale = np.float32(1.0 / math.sqrt(HD))
    cos, sins = rope_tables()
    mask = causal_mask_tile()
    in_maps = []
    for c in range(8):
        b, g = c // 4, c % 4
        in_maps.append(
            {
                "xt": np.ascontiguousarray(x[b].T),
                "wq": np.ascontiguousarray(Wq[:, 512 * g : 512 * (g + 1)]) * scale,
                "wk": np.ascontiguousarray(Wk[:, 128 * g : 128 * (g + 1)]),
                "wv": np.ascontiguousarray(Wv[:, 128 * g : 128 * (g + 1)]),
                "wo": np.ascontiguousarray(Wo[512 * g : 512 * (g + 1), :]),
                "cost": cos,
                "sint": sins,
                "maskT": mask,
                "identd": np.eye(128, dtype=np.float32),
                "swapd": half_swap_tile(),
            }
        )
    return in_maps


_CACHE = {}


def _get_nc():
    if "nc" not in _CACHE:
        _CACHE["nc"] = build_nc()
    return _CACHE["nc"]


def kernel(**inputs):
    x = np.asarray(inputs["x"], np.float32)
    Wq = np.asarray(inputs["Wq"], np.float32)
    Wk = np.asarray(inputs["Wk"], np.float32)
    Wv = np.asarray(inputs["Wv"], np.float32)
    Wo = np.asarray(inputs["Wo"], np.float32)
    in_maps = make_in_maps(x, Wq, Wk, Wv, Wo)
    nc = _get_nc()
    res = run_bass_kernel_spmd(nc, in_maps, core_ids=list(range(8)))
    outs = [r["y"] for r in res.results]
    y = np.stack(
        [
            outs[0] + outs[1] + outs[2] + outs[3],
            outs[4] + outs[5] + outs[6] + outs[7],
        ]
    )
    return y.astype(np.float32)
